# revision 16
# baseline (speedup 1.0000x reference)
"""Trainium2 Bass kernel for nn_Attention (B=2, N=2048, D=1024, H=16, hd=32).

Sharding: core c = (b, qc) with b = c//4, qc = c%4 — data parallel on batch,
sequence parallel on 512-query blocks. Each core computes K/V for ALL 16
heads over the full sequence (4x recompute vs head sharding) plus Q for its
own 512 queries, then full attention + projection for those queries. Output
rows are disjoint across cores, so there are NO collectives at all.

The host rotates x's key columns per core so the core's query block always
sits at columns [0:512] of its x copy (softmax sums over keys, so key order
is irrelevant as long as K and V share it).

All matmuls run bf16 with fp32 PSUM accumulation; the attention scale is
folded into W_q on the host. Softmax is max-free (logits are O(0.2) here).
exp is split between ScalarE (table exp) and VectorE (single-pass
Schraudolph: bf16 bits of exp(z) ~= round(z*128/ln2 + B) computed as a
tensor_scalar mult+add straight into an int16 tile aliased as bf16).
Denominators accumulate via ones-vector matmuls col-packed into the PE
array; reciprocals via exp(-ln(d)) on ScalarE (both functions live in one
ACT table set).
"""

import os

import numpy as np
import ml_dtypes

import concourse.bass as bass
import concourse.bacc as bacc
import concourse.mybir as mybir
import concourse.tile as tile
from concourse.bass_utils import run_bass_kernel_spmd

B, N, D = 2, 2048, 1024
H, HD, CD = 16, 32, 512
NCORES = 8
NQ = 512                            # queries per core
SCALE = (64 ** -0.5) / (0.5 ** 0.5)
BF = mybir.dt.bfloat16
F32 = mybir.dt.float32
I16 = mybir.dt.int16
AF = mybir.ActivationFunctionType
ALU = mybir.AluOpType

DT = D // 128                       # 8 contraction tiles over model dim
NKT = N // 128                      # 16 key tiles
A_FE = 184.6649652337873            # 128 / ln(2)
B_FE = 16250.65                     # bits offset (robust to rint/floor)
FE_MOD, FE_PHASE = 3, 2             # unit u on DVE fast-exp iff u%MOD==PHASE
DBG_NO_FE = bool(os.environ.get("DBG_NO_FE"))        # all exp on ScalarE
DBG_NO_DIAG = bool(os.environ.get("DBG_NO_DIAG"))    # bc via partition-0 rows
DBG_FULL_XDMA = bool(os.environ.get("DBG_FULL_XDMA"))  # single x DMA


def _emit_v_pair(nc, stp, v_sb, x_sb, wv_sb, tp):
    """V for key tiles tp, tp+1 into one [128,1024] psum tile."""
    acc = stp.tile([128, 1024], F32, tag="st", name=f"v{tp}")
    for i in range(2):
        t = tp + i
        for dt in range(DT):
            nc.tensor.matmul(
                acc[:, 512 * i:512 * (i + 1)],
                x_sb[:, dt, 128 * t:128 * (t + 1)],
                wv_sb[:, dt, :],
                start=(dt == 0), stop=(dt == DT - 1),
            )
    for i in range(2):
        nc.vector.tensor_copy(v_sb[:, tp + i, :], acc[:, 512 * i:512 * (i + 1)])


def _emit_k_half(nc, stp, kt_sb, x_sb, wk_sb, g, kp):
    """K^T for group g, key blocks kp, kp+1 into one [128,1024] psum tile."""
    acc = stp.tile([128, 1024], F32, tag="st", name=f"k{g}_{kp}")
    for i in range(2):
        kb = kp + i
        for dt in range(DT):
            nc.tensor.matmul(
                acc[:, 512 * i:512 * (i + 1)],
                wk_sb[:, dt, 128 * g:128 * (g + 1)],
                x_sb[:, dt, 512 * kb:512 * (kb + 1)],
                start=(dt == 0), stop=(dt == DT - 1),
            )
    for i in range(2):
        nc.vector.tensor_copy(
            kt_sb[:, g, 512 * (kp + i):512 * (kp + i + 1)],
            acc[:, 512 * i:512 * (i + 1)])


def build_nc():
    nc = bacc.Bacc(num_devices=NCORES)

    xT = nc.dram_tensor("xT", [D, N], BF, kind="ExternalInput")
    wqT = nc.dram_tensor("wqT", [D, CD], BF, kind="ExternalInput")
    wkT = nc.dram_tensor("wkT", [D, CD], BF, kind="ExternalInput")
    wvT = nc.dram_tensor("wvT", [D, CD], BF, kind="ExternalInput")
    wpT = nc.dram_tensor("wpT", [CD, D], BF, kind="ExternalInput")
    biasT = nc.dram_tensor("biasT", [128, 8], F32, kind="ExternalInput")
    out = nc.dram_tensor("out", [D, NQ], F32, kind="ExternalOutput")

    with tile.TileContext(nc) as tc:
        with (
            tc.tile_pool(name="wp", bufs=1) as wp,
            tc.tile_pool(name="sp", bufs=2) as sp,
            tc.tile_pool(name="ptp", bufs=5) as ptp,
            tc.tile_pool(name="obp", bufs=2) as obp,
        ):
            # ---- input DMA (queries of this core live at x cols 0:512)
            wq_sb = wp.tile([128, DT, CD], BF)
            nc.sync.dma_start(wq_sb[:], wqT[:].rearrange("(a p) n -> p a n", p=128))
            x_sb = wp.tile([128, DT, N], BF)
            if DBG_FULL_XDMA:
                nc.sync.dma_start(
                    x_sb[:], xT[:].rearrange("(a p) n -> p a n", p=128))
            else:
                nc.sync.dma_start(
                    x_sb[:, :, 0:512],
                    xT[:, 0:512].rearrange("(a p) n -> p a n", p=128))
            wk_sb = wp.tile([128, DT, CD], BF)
            nc.sync.dma_start(wk_sb[:], wkT[:].rearrange("(a p) n -> p a n", p=128))
            wv_sb = wp.tile([128, DT, CD], BF)
            nc.sync.dma_start(wv_sb[:], wvT[:].rearrange("(a p) n -> p a n", p=128))
            if not DBG_FULL_XDMA:
                for cb in range(1, 4):
                    nc.sync.dma_start(
                        x_sb[:, :, 512 * cb:512 * (cb + 1)],
                        xT[:, 512 * cb:512 * (cb + 1)].rearrange(
                            "(a p) n -> p a n", p=128))
            wp_sb = wp.tile([128, 4, D], BF)
            nc.sync.dma_start(wp_sb[:], wpT[:].rearrange("(a p) n -> p a n", p=128))
            bias_sb = wp.tile([128, 8], F32)
            nc.sync.dma_start(bias_sb[:], biasT[:])

            gpb_a = wp.tile([128, 1024], BF)
            nc.gpsimd.memset(gpb_a[:], 1.0)
            gpb_b = wp.tile([128, 1024], F32)
            nc.gpsimd.tensor_copy(gpb_b[:], gpb_a[:])
            nc.gpsimd.tensor_tensor(gpb_b[:], gpb_b[:], gpb_a[:], ALU.add)
            nc.gpsimd.tensor_tensor(gpb_b[:], gpb_b[:], gpb_a[:], ALU.add)

            ones_sb = wp.tile([128, 32], BF)
            nc.vector.memset(ones_sb[:], 1.0)
            ones32_sb = wp.tile([128, 32], F32)
            nc.vector.memset(ones32_sb[:], 1.0)

            qt_sb = wp.tile([128, 4, NQ], BF)
            kt_sb = wp.tile([128, 4, N], BF)
            v_sb = wp.tile([128, NKT, CD], BF)
            ot_sb = wp.tile([128, 4, NQ], BF)

            with (
                tc.tile_pool(name="stp", bufs=3, space=bass.MemorySpace.PSUM) as stp,
                tc.tile_pool(name="oap", bufs=1, space=bass.MemorySpace.PSUM) as oap,
                tc.tile_pool(name="sdp", bufs=1, space=bass.MemorySpace.PSUM) as sdp,
            ):
                # ---- Q^T for this core's 512 queries (2 groups per psum tile)
                for gp in range(2):
                    acc = stp.tile([128, 1024], F32, tag="st", name=f"q{gp}")
                    for i in range(2):
                        g = 2 * gp + i
                        for dt in range(DT):
                            nc.tensor.matmul(
                                acc[:, 512 * i:512 * (i + 1)],
                                wq_sb[:, dt, 128 * g:128 * (g + 1)],
                                x_sb[:, dt, 0:512],
                                start=(dt == 0), stop=(dt == DT - 1),
                            )
                    for i in range(2):
                        nc.vector.tensor_copy(
                            qt_sb[:, 2 * gp + i, :], acc[:, 512 * i:512 * (i + 1)])

                # ---- K^T group 0, first V pairs
                _emit_k_half(nc, stp, kt_sb, x_sb, wk_sb, 0, 0)
                _emit_k_half(nc, stp, kt_sb, x_sb, wk_sb, 0, 2)
                _emit_v_pair(nc, stp, v_sb, x_sb, wv_sb, 0)

                # ---- attention, group-sequential, software-pipelined by
                # one key-tile: AV/den for t-1 are emitted after QK of t so
                # the tensor queue never head-of-line blocks on exp
                uidx = 0
                for g in range(4):
                    o_acc = oap.tile([128, NQ], F32, tag="o", name=f"o{g}")
                    s_acc = sdp.tile([128, NQ], F32, tag="s", name=f"s{g}")

                    def emit_avden(pts, t, g=g, o_acc=o_acc, s_acc=s_acc):
                        for h in range(4):
                            rhs = pts[h // 2][:, 512 * (h % 2):512 * (h % 2 + 1)]
                            nc.tensor.matmul(
                                o_acc[32 * h:32 * (h + 1), :],
                                v_sb[:, t,
                                     128 * g + 32 * h:128 * g + 32 * (h + 1)],
                                rhs,
                                start=(t == 0), stop=(t == NKT - 1),
                                tile_position=(0, 32 * h),
                                skip_group_check=(h > 0),
                            )
                        for h in range(4):
                            rhs = pts[h // 2][:, 512 * (h % 2):512 * (h % 2 + 1)]
                            nc.tensor.matmul(
                                s_acc[32 * h:32 * (h + 1), :],
                                ones_sb[:],
                                rhs,
                                start=(t == 0), stop=(t == NKT - 1),
                                tile_position=(0, 32 * h),
                                skip_group_check=(h > 0),
                            )

                    prev = None
                    for t in range(NKT):
                        sts = []
                        for p in range(2):
                            stx = stp.tile([128, 1024], F32, tag="st",
                                           name=f"st{g}_{t}_{p}")
                            sts.append(stx)
                            for i in range(2):
                                h = 2 * p + i
                                tp = (96, 0) if h == 3 else None
                                nc.tensor.matmul(
                                    stx[:, 512 * i:512 * (i + 1)],
                                    kt_sb[32 * h:32 * (h + 1), g,
                                          128 * t:128 * (t + 1)],
                                    qt_sb[32 * h:32 * (h + 1), g, :],
                                    start=True, stop=True, tile_position=tp,
                                )
                        pts = []
                        for p in range(2):
                            pt = ptp.tile([128, 1024], BF, tag="pt")
                            pts.append(pt)
                            if (uidx % FE_MOD == FE_PHASE and t < 14
                                    and not DBG_NO_FE):
                                nc.vector.tensor_scalar(
                                    pt[:].bitcast(I16), sts[p][:], A_FE, B_FE,
                                    ALU.mult, ALU.add)
                            else:
                                nc.scalar.activation(pt[:], sts[p][:], AF.Exp)
                            uidx += 1
                        # fillers: keep the PE warm while exp runs
                        if g == 0:
                            if t % 2 == 0 and t < 14:
                                _emit_v_pair(nc, stp, v_sb, x_sb, wv_sb, t + 2)
                            elif t == 1:
                                _emit_k_half(nc, stp, kt_sb, x_sb, wk_sb, 1, 0)
                            elif t == 3:
                                _emit_k_half(nc, stp, kt_sb, x_sb, wk_sb, 1, 2)
                        elif g < 3 and t in (1, 9):
                            _emit_k_half(nc, stp, kt_sb, x_sb, wk_sb, g + 1,
                                         0 if t == 1 else 2)
                        if prev is not None:
                            emit_avden(prev, t - 1)
                        prev = pts
                    emit_avden(prev, NKT - 1)

                    # normalize: 1/sum on DVE straight from PSUM, band
                    # broadcast on (otherwise idle) GpSimd, one multiply
                    s_sb = sp.tile([128, NQ], F32, tag="ssb", name=f"ssb{g}")
                    nc.vector.tensor_copy(s_sb[:], s_acc[:])
                    r_sb = sp.tile([128, NQ], F32, tag="rsb", name=f"rsb{g}")
                    nc.vector.reciprocal_approx_fast(r_sb[:], s_sb[:])
                    bc = stp.tile([128, 1024], F32, tag="st", name=f"bc{g}")
                    for h in range(4):
                        nc.tensor.matmul(
                            bc[32 * h:32 * (h + 1), 0:512],
                            ones32_sb[32 * h:32 * h + 1, :],
                            r_sb[32 * h:32 * h + 1, :],
                            start=True, stop=True,
                            tile_position=(32 * h, 32 * h),
                        )
                    bc_sb = sp.tile([128, NQ], F32, tag="bcs", name=f"bcs{g}")
                    nc.vector.tensor_copy(bc_sb[:], bc[:, 0:512])
                    nc.vector.tensor_tensor(
                        ot_sb[:, g, :], o_acc[:], bc_sb[:], ALU.mult)

                # ---- projection: out^T[e, q] = wpT.T @ ot, + bias
                for ep in range(4):
                    pj = stp.tile([128, 1024], F32, tag="st", name=f"pj{ep}")
                    for i in range(2):
                        et = 2 * ep + i
                        for ct in range(4):
                            nc.tensor.matmul(
                                pj[:, 512 * i:512 * (i + 1)],
                                wp_sb[:, ct, 128 * et:128 * (et + 1)],
                                ot_sb[:, ct, :],
                                start=(ct == 0), stop=(ct == 3),
                            )
                    for i in range(2):
                        et = 2 * ep + i
                        ob = obp.tile([128, NQ], F32, tag="ob")
                        nc.vector.tensor_scalar(
                            ob[:], pj[:, 512 * i:512 * (i + 1)],
                            bias_sb[:, et:et + 1], None, ALU.add)
                        nc.sync.dma_start(out[128 * et:128 * (et + 1), :], ob[:])
    nc.compile()
    return nc


_NC = None


def kernel(x, w_qkv, w_proj, b_proj):
    global _NC
    if _NC is None:
        _NC = build_nc()
    bf = ml_dtypes.bfloat16

    wq = np.ascontiguousarray((w_qkv[:CD] * SCALE).T).astype(bf)       # [D, CD]
    wk = np.ascontiguousarray(w_qkv[CD:2 * CD].T).astype(bf)
    wv = np.ascontiguousarray(w_qkv[2 * CD:3 * CD].T).astype(bf)
    wpt = np.ascontiguousarray(w_proj[:, :CD].T).astype(bf)            # [CD, D]
    biasT = np.ascontiguousarray(
        b_proj.astype(np.float32).reshape(8, 128).T)                   # [128, 8]

    xTs = [np.ascontiguousarray(x[b].T).astype(bf) for b in range(B)]  # [D, N]

    in_maps = []
    for c in range(NCORES):
        b, qc = c // 4, c % 4
        xr = np.concatenate([xTs[b][:, 512 * qc:], xTs[b][:, :512 * qc]], axis=1)
        in_maps.append({
            "xT": np.ascontiguousarray(xr),
            "wqT": wq, "wkT": wk, "wvT": wv, "wpT": wpt,
            "biasT": biasT,
        })

    trace = bool(os.environ.get("KERNEL_TRACE"))
    rr = run_bass_kernel_spmd(
        _NC, in_maps, list(range(NCORES)),
        trace=trace, tmpdir=os.environ.get("KERNEL_TRACE_DIR") or None,
    )
    if rr.exec_time_ns is not None:
        print(f"HW exec time: {rr.exec_time_ns} ns")
    res = rr.results

    out = np.empty((B, N, D), dtype=np.float32)
    for c in range(NCORES):
        b, qc = c // 4, c % 4
        out[b, 512 * qc:512 * (qc + 1), :] = res[c]["out"].T
    return out
